# revision 2
# baseline (speedup 1.0000x reference)
"""MLDecoder classification head on 8 Trainium2 NeuronCores.

Sharding: data-parallel over batch B=64 -> 8 cores x 8 batches, all params
replicated. Inside each core a per-batch pipeline computes
  mem = relu(x @ We) ; cross-attention(q(query_embed), mem) ; FFN ; LN
and a final grouped-FC phase computes logits for the core's 8 batches.

Host-side prep is limited to layout transforms (transposes / reshapes /
sharding) and standard parameter folding (LN gains into adjacent weights,
softmax scale into wq, zero-sum bias absorption), all O(params).
"""

import numpy as np
import ml_dtypes

import concourse.bass as bass
import concourse.mybir as mybir
import concourse.tile as tile
from concourse import bacc
from concourse.masks import make_identity
from concourse.bass_utils import run_bass_kernel_spmd

# ---------------- problem dims (hardcoded) ----------------
B, C_IN, H, W = 64, 2048, 14, 14
D, FF, G, NCLS, NH = 768, 2048, 100, 9605, 8
DF = 97
HD = D // NH  # 96
S = H * W  # 196
EPS = 1e-5

N_CORES = 8
BL = B // N_CORES  # 8 batches per core

MODE = "bf16"  # "bf16" | "fp32r" | "fp32"

KC_C = C_IN // 128  # 16
KC_D = D // 128  # 6
KC_FF = FF // 128  # 16
NGRP = 8  # dup_pool groups per streamed chunk
NCHUNK_W = NGRP * DF  # 485

f32 = mybir.dt.float32
bf16 = mybir.dt.bfloat16


def _mode_cfg(mode):
    if mode == "bf16":
        return mybir.dt.bfloat16, ml_dtypes.bfloat16, S
    if mode == "fp32r":
        return mybir.dt.float32r, np.float32, 256
    if mode == "fp32":
        return mybir.dt.float32, np.float32, S
    raise ValueError(mode)


MM_DT, NP_DT, SP = _mode_cfg(MODE)
N_SBLK = (SP + 127) // 128
SBLK_ROWS = [min(128, SP - 128 * i) for i in range(N_SBLK)]


# ---------------- device kernel ----------------

def build_kernel():
    nc = bacc.Bacc("TRN2", target_bir_lowering=False)

    dm = MM_DT
    specs = [
        ("x", (BL, C_IN, SP), dm), ("wembT", (C_IN, D), dm),
        ("wkT", (D, D), dm), ("wvT", (D, D), dm), ("wqT", (D, D), dm),
        ("woT", (D, D), dm), ("w1T", (D, FF), dm), ("w2T", (FF, D), dm),
        ("dup", (D, G, DF), bf16), ("db2", (G, DF), bf16),
        ("qe", (G, D), f32), ("g1", (D,), f32), ("be1", (D,), f32),
        ("g2", (D,), f32), ("bemb", (D,), f32), ("bk", (D,), f32),
        ("bq", (D,), f32), ("bl1", (FF,), f32),
        ("bo_row", (1, D), dm), ("bl2_row", (1, D), dm),
        ("ones_mm", (1, 128), dm), ("ones_bf", (1, 128), bf16),
    ]
    hs = {n: nc.dram_tensor(n, shp, dt, kind="ExternalInput") for n, shp, dt in specs}
    hs["out"] = nc.dram_tensor("out", (BL, NCLS), f32, kind="ExternalOutput")

    with tile.TileContext(nc) as tc:
        _body(nc, tc, hs)
    nc.finalize()
    return nc


def _body(nc, tc, hs):
    from contextlib import ExitStack

    def dram(name):
        return hs[name][:]

    ctx = ExitStack()
    with ctx:
        const = ctx.enter_context(tc.tile_pool(name="const", bufs=1))
        dm = MM_DT

        # ---- small constants first (phase0 critical path), then big weights ----
        bemb = const.tile([128, KC_D], f32)
        nc.sync.dma_start(out=bemb, in_=dram("bemb").rearrange("(c p) -> p c", p=128))
        bk = const.tile([96, NH], f32)
        nc.sync.dma_start(out=bk, in_=dram("bk").rearrange("(h p) -> p h", p=96))
        bq = const.tile([96, NH], f32)
        nc.sync.dma_start(out=bq, in_=dram("bq").rearrange("(h p) -> p h", p=96))
        bl1 = const.tile([128, KC_FF], f32)
        nc.sync.dma_start(out=bl1, in_=dram("bl1").rearrange("(c p) -> p c", p=128))
        bo_row = const.tile([1, D], dm)
        nc.sync.dma_start(out=bo_row, in_=dram("bo_row"))
        bl2_row = const.tile([1, D], dm)
        nc.sync.dma_start(out=bl2_row, in_=dram("bl2_row"))
        ones_mm = const.tile([1, 128], dm)
        nc.sync.dma_start(out=ones_mm, in_=dram("ones_mm"))
        ones_bf = const.tile([1, 128], bf16)
        nc.sync.dma_start(out=ones_bf, in_=dram("ones_bf"))

        def bcast(ap, n_part):
            return bass.AP(tensor=ap.tensor, offset=ap.offset,
                           ap=[[0, n_part]] + list(ap.ap))

        g2b = const.tile([G, D], f32)
        nc.sync.dma_start(out=g2b, in_=bcast(dram("g2"), G))

        # big weights: embed first, then attention, then FFN weights on the
        # scalar queue (needed latest). wqT lives in the scoped phase-0 pool.
        wembT = const.tile([128, KC_C, D], dm)
        wv_emb = dram("wembT").rearrange("(kc p) d -> p kc d", p=128)
        for kc in range(KC_C):
            nc.sync.dma_start(out=wembT[:, kc, :], in_=wv_emb[:, kc, :])
        wkT = const.tile([128, KC_D, D], dm)
        nc.sync.dma_start(out=wkT, in_=dram("wkT").rearrange("(kc p) e -> p kc e", p=128))
        wvT = const.tile([128, KC_D, D], dm)
        nc.sync.dma_start(out=wvT, in_=dram("wvT").rearrange("(kc p) e -> p kc e", p=128))
        # out-proj consumes k (=d) in 96-row blocks matching ctxT head tiles
        woT = const.tile([96, NH, D], dm)
        nc.sync.dma_start(out=woT, in_=dram("woT").rearrange("(h p) e -> p h e", p=96))
        w1T = const.tile([128, KC_D, FF], dm)
        nc.scalar.dma_start(out=w1T, in_=dram("w1T").rearrange("(kc p) f -> p kc f", p=128))
        w2T = const.tile([128, KC_FF, D], dm)
        nc.scalar.dma_start(out=w2T, in_=dram("w2T").rearrange("(kc p) e -> p kc e", p=128))

        ident = const.tile([128, 128], f32)
        make_identity(nc, ident)
        eps_t = const.tile([128, 1], f32)
        nc.vector.memset(eps_t, EPS)

        smal = ctx.enter_context(tc.tile_pool(name="smal", bufs=4))
        ps = ctx.enter_context(tc.tile_pool(name="ps", bufs=8, space="PSUM"))

        # ---------- helpers ----------
        def layernorm_core(out_sb, in_sb, tag):
            """out = (in - mean)/sqrt(var+EPS), rows [G, D]."""
            st = smal.tile([G, 3, 6], f32, tag=tag + "_st")
            iv = in_sb.rearrange("g (n f) -> g n f", f=256)
            for i in range(3):
                nc.vector.bn_stats(out=st[:, i, :], in_=iv[:, i, :])
            mv = smal.tile([G, 2], f32, tag=tag + "_mv")
            nc.vector.bn_aggr(out=mv, in_=st)
            sd = smal.tile([G, 1], f32, tag=tag + "_sd")
            nc.scalar.activation(out=sd, in_=mv[:, 1:2],
                                 func=mybir.ActivationFunctionType.Sqrt,
                                 bias=eps_t[:G], scale=1.0)
            nc.vector.reciprocal(out=sd, in_=sd)
            nc.vector.tensor_scalar(out=out_sb, in0=in_sb,
                                    scalar1=mv[:, 0:1], scalar2=sd,
                                    op0=mybir.AluOpType.subtract,
                                    op1=mybir.AluOpType.mult)

        def transpose_cols(dst_ap, src_sb, kc, rows=128):
            """dst (=[rows, P]) = src[:, kc*128 : kc*128+rows].T ; dst dtype rounds."""
            p_t = ps.tile([128, 512], f32, tag="ps")
            pn = src_sb.shape[0]
            nc.tensor.transpose(p_t[:rows, :pn], src_sb[:, kc * 128:kc * 128 + rows],
                                ident[:pn, :pn])
            nc.scalar.activation(out=dst_ap, in_=p_t[:rows, :pn],
                                 func=mybir.ActivationFunctionType.Copy,
                                 bias=0.0, scale=1.0)

        # ---------- phase 0: tgt_n and qT (batch independent) ----------
        tgt_n = const.tile([G, D], f32)
        tnT = const.tile([128, KC_D, G], dm)
        qT = const.tile([96, NH, G], dm)
        with tc.tile_pool(name="tmp0", bufs=1) as tmp0:
            wqT = tmp0.tile([128, KC_D, D], dm)
            nc.sync.dma_start(out=wqT, in_=dram("wqT").rearrange("(kc p) e -> p kc e", p=128))
            qe_sb = tmp0.tile([G, D], f32)
            nc.sync.dma_start(out=qe_sb, in_=dram("qe"))
            g1b = tmp0.tile([G, D], f32)
            nc.sync.dma_start(out=g1b, in_=bcast(dram("g1"), G))
            be1b = tmp0.tile([G, D], f32)
            nc.sync.dma_start(out=be1b, in_=bcast(dram("be1"), G))
            qe2 = tmp0.tile([G, D], f32)
            nc.scalar.mul(qe2, qe_sb, 2.0)
            tnc = tmp0.tile([G, D], f32)
            layernorm_core(tnc, qe2, "ln1")
            nc.vector.tensor_mul(tgt_n, tnc, g1b)
            nc.vector.tensor_add(tgt_n, tgt_n, be1b)

            for kc in range(KC_D):
                transpose_cols(tnT[:, kc, :], tgt_n, kc)

            for h in range(NH):
                p_q = ps.tile([128, 512], f32, tag="ps")
                for kc in range(KC_D):
                    nc.tensor.matmul(p_q[:96, :G], wqT[:, kc, h * 96:(h + 1) * 96],
                                     tnT[:, kc, :], start=(kc == 0), stop=(kc == KC_D - 1))
                nc.vector.tensor_scalar(out=qT[:, h, :], in0=p_q[:96, :G],
                                        scalar1=bq[:, h:h + 1], scalar2=None,
                                        op0=mybir.AluOpType.add)

        work = ctx.enter_context(tc.tile_pool(name="work", bufs=2))
        xpool = ctx.enter_context(tc.tile_pool(name="xpool", bufs=20))
        lnpool = ctx.enter_context(tc.tile_pool(name="lnpool", bufs=3))
        hpool = ctx.enter_context(tc.tile_pool(name="hpool", bufs=1))
        dupp = ctx.enter_context(tc.tile_pool(name="dupp", bufs=2))

        # ---------- per-batch pipeline (two-stage software pipeline) ----------
        hT = hpool.tile([128, KC_D, BL, G], bf16)
        x_view = dram("x").rearrange("b (kc p) s -> b p kc s", p=128)
        state = {}

        def emit_x(b):
            xs = []
            for kc in range(KC_C):
                x_kc = xpool.tile([128, SP], dm, tag="x")
                nc.gpsimd.dma_start(out=x_kc, in_=x_view[b][:, kc, :])
                xs.append(x_kc)
            return xs

        def emit_embed(b, xs):
            # embed: memT[d, s] = relu(sum_c wembT[c, d] * x[c, s] + bemb)
            memT = work.tile([128, KC_D, SP], dm, tag="memT")
            for dblk in range(KC_D):
                p_e = ps.tile([128, 512], f32, tag="ps")
                for kc in range(KC_C):
                    nc.tensor.matmul(p_e[:, :SP], wembT[:, kc, dblk * 128:(dblk + 1) * 128],
                                     xs[kc], start=(kc == 0), stop=(kc == KC_C - 1))
                nc.scalar.activation(out=memT[:, dblk, :], in_=p_e[:, :SP],
                                     func=mybir.ActivationFunctionType.Relu,
                                     bias=bemb[:, dblk:dblk + 1], scale=1.0)
            return memT

        def emit_kT(b, memT):
            kT = work.tile([96, NH, SP], dm, tag="kT")
            for h in range(NH):
                p_k = ps.tile([128, 512], f32, tag="ps")
                for kc in range(KC_D):
                    nc.tensor.matmul(p_k[:96, :SP], wkT[:, kc, h * 96:(h + 1) * 96],
                                     memT[:, kc, :], start=(kc == 0), stop=(kc == KC_D - 1))
                nc.vector.tensor_scalar(out=kT[:, h, :], in0=p_k[:96, :SP],
                                        scalar1=bk[:, h:h + 1], scalar2=None,
                                        op0=mybir.AluOpType.add)
            return kT

        def emit_v(b, memT):
            v_sb = work.tile([128, N_SBLK, D], dm, tag="v")
            for sblk in range(N_SBLK):
                rows = SBLK_ROWS[sblk]
                for nch in range(2):
                    p_v = ps.tile([128, 512], f32, tag="ps")
                    for kc in range(KC_D):
                        nc.tensor.matmul(
                            p_v[:rows, :384],
                            memT[:, kc, sblk * 128:sblk * 128 + rows],
                            wvT[:, kc, nch * 384:(nch + 1) * 384],
                            start=(kc == 0), stop=(kc == KC_D - 1))
                    nc.vector.tensor_copy(
                        out=v_sb[:rows, sblk, nch * 384:(nch + 1) * 384],
                        in_=p_v[:rows, :384])
            return v_sb

        def emit_attn(b, kT, v_sb):
            ctxT = work.tile([96, NH, G], dm, tag="ctxT")
            for h in range(NH):
                p_s = ps.tile([128, 512], f32, tag="ps")
                nc.tensor.matmul(p_s[:G, :SP], qT[:, h, :], kT[:, h, :],
                                 start=True, stop=True)
                if SP > S:
                    nc.vector.memset(p_s[:G, S:SP], -1e30)
                negmax = smal.tile([G, 1], f32, tag="negmax")
                nc.vector.reduce_max(negmax, p_s[:G, :SP],
                                     axis=mybir.AxisListType.X, negate=True)
                p_sb = work.tile([G, SP], f32, tag="p_sb")
                rsum = smal.tile([G, 1], f32, tag="rsum")
                nc.scalar.activation(out=p_sb, in_=p_s[:G, :SP],
                                     func=mybir.ActivationFunctionType.Exp,
                                     bias=negmax, scale=1.0, accum_out=rsum)
                nc.vector.reciprocal(out=rsum, in_=rsum)
                nc.vector.tensor_scalar_mul(p_sb, p_sb, rsum)
                attnT = work.tile([128, N_SBLK, G], dm, tag="attnT")
                for sblk in range(N_SBLK):
                    transpose_cols(attnT[:SBLK_ROWS[sblk], sblk, :], p_sb, sblk,
                                   rows=SBLK_ROWS[sblk])
                p_c = ps.tile([128, 512], f32, tag="ps")
                for sblk in range(N_SBLK):
                    rows = SBLK_ROWS[sblk]
                    nc.tensor.matmul(p_c[:96, :G],
                                     v_sb[:rows, sblk, h * 96:(h + 1) * 96],
                                     attnT[:rows, sblk, :],
                                     start=(sblk == 0), stop=(sblk == N_SBLK - 1))
                nc.vector.tensor_copy(out=ctxT[:, h, :], in_=p_c[:96, :G])
            return ctxT

        def emit_oproj_ln2(b, ctxT):
            t2 = lnpool.tile([G, D], f32, tag="lnbuf")
            for nch in range(2):
                p_o = ps.tile([128, 512], f32, tag="ps")
                nc.tensor.matmul(p_o[:G, :384], ones_mm[:, :G],
                                 bo_row[:, nch * 384:(nch + 1) * 384],
                                 start=True, stop=False)
                for h in range(NH):
                    nc.tensor.matmul(p_o[:G, :384], ctxT[:, h, :],
                                     woT[:, h, nch * 384:(nch + 1) * 384],
                                     start=False, stop=(h == NH - 1))
                nc.vector.tensor_add(t2[:, nch * 384:(nch + 1) * 384],
                                     tgt_n[:, nch * 384:(nch + 1) * 384],
                                     p_o[:G, :384])
            lnc2 = lnpool.tile([G, D], f32, tag="lnbuf")
            layernorm_core(lnc2, t2, "ln2")
            lnc2T = work.tile([128, KC_D, G], dm, tag="lnc2T")
            for kc in range(KC_D):
                transpose_cols(lnc2T[:, kc, :], lnc2, kc)
            return t2, lnc2, lnc2T

        def emit_ffn1(b, lnc2T):
            ffT = work.tile([128, KC_FF, G], dm, tag="ffT")
            for fblk in range(KC_FF):
                p_f = ps.tile([128, 512], f32, tag="ps")
                for kc in range(KC_D):
                    nc.tensor.matmul(p_f[:, :G], w1T[:, kc, fblk * 128:(fblk + 1) * 128],
                                     lnc2T[:, kc, :], start=(kc == 0), stop=(kc == KC_D - 1))
                nc.scalar.activation(out=ffT[:, fblk, :], in_=p_f[:, :G],
                                     func=mybir.ActivationFunctionType.Relu,
                                     bias=bl1[:, fblk:fblk + 1], scale=1.0)
            return ffT

        def emit_ffn2_ln3(b, t2, lnc2, ffT):
            for nch in range(2):
                p_2 = ps.tile([128, 512], f32, tag="ps")
                nc.tensor.matmul(p_2[:G, :384], ones_mm[:, :G],
                                 bl2_row[:, nch * 384:(nch + 1) * 384],
                                 start=True, stop=False)
                for kc in range(KC_FF):
                    nc.tensor.matmul(p_2[:G, :384], ffT[:, kc, :],
                                     w2T[:, kc, nch * 384:(nch + 1) * 384],
                                     start=False, stop=(kc == KC_FF - 1))
                sl = slice(nch * 384, (nch + 1) * 384)
                nc.vector.tensor_mul(t2[:, sl], lnc2[:, sl], g2b[:, sl])
                nc.vector.tensor_add(t2[:, sl], t2[:, sl], p_2[:G, :384])
            lnc3 = lnpool.tile([G, D], f32, tag="lnbuf")
            layernorm_core(lnc3, t2, "ln3")
            return lnc3

        def emit_hT(b, lnc3):
            for kc in range(KC_D):
                transpose_cols(hT[:, kc, b, :], lnc3, kc)

        # interleaved emission: stage-2 of batch b-1 woven between
        # stage-1 pieces of batch b so LN/softmax chains hide under PE work
        xs_next = emit_x(0)
        for b in range(BL):
            xs = xs_next
            if b + 1 < BL:
                xs_next = emit_x(b + 1)
            prev = state.pop(b - 1, None)
            if prev is not None:
                t2p, lnc2p, lnc2Tp = emit_oproj_ln2(b - 1, prev)
            memT = emit_embed(b, xs)
            if prev is not None:
                ffTp = emit_ffn1(b - 1, lnc2Tp)
            kT = emit_kT(b, memT)
            if prev is not None:
                lnc3p = emit_ffn2_ln3(b - 1, t2p, lnc2p, ffTp)
            v_sb = emit_v(b, memT)
            if prev is not None:
                emit_hT(b - 1, lnc3p)
            state[b] = emit_attn(b, kT, v_sb)

        bl = BL - 1
        ctxT_l = state.pop(bl)
        t2l, lnc2l, lnc2Tl = emit_oproj_ln2(bl, ctxT_l)
        ffTl = emit_ffn1(bl, lnc2Tl)
        lnc3l = emit_ffn2_ln3(bl, t2l, lnc2l, ffTl)
        emit_hT(bl, lnc3l)

        # ---------- grouped FC ----------
        dup_view = dram("dup").rearrange("(kc p) g f -> p kc g f", p=128)
        out_flat = dram("out")
        g0 = 0
        while g0 < G:
            ng = min(NGRP, G - g0)
            dup_sb = dupp.tile([128, KC_D, NGRP, DF], bf16, tag="dup")
            gsl = slice(g0, g0 + ng)
            nc.scalar.dma_start(out=dup_sb[:, :KC_D // 2, :ng, :],
                                in_=dup_view[:, :KC_D // 2, gsl, :])
            nc.sync.dma_start(out=dup_sb[:, KC_D // 2:, :ng, :],
                              in_=dup_view[:, KC_D // 2:, gsl, :])
            db2_sb = dupp.tile([1, NGRP, DF], bf16, tag="db2c")
            nc.sync.dma_start(out=db2_sb[:, :ng, :], in_=dram("db2")[gsl, :])
            lchunk = work.tile([BL, NGRP, DF], f32, tag="lchunk")
            for gi in range(ng):
                g = g0 + gi
                p_g = ps.tile([128, 512], f32, tag="ps")
                nc.tensor.matmul(p_g[:BL, :DF], ones_bf[:, :BL],
                                 db2_sb[:, gi, :], start=True, stop=False)
                for kc in range(KC_D):
                    nc.tensor.matmul(p_g[:BL, :DF], hT[:, kc, :, g],
                                     dup_sb[:, kc, gi, :],
                                     start=False, stop=(kc == KC_D - 1))
                nc.vector.tensor_copy(out=lchunk[:, gi, :], in_=p_g[:BL, :DF])
            c0 = g0 * DF
            wout = min(ng * DF, NCLS - c0)
            nc.gpsimd.dma_start(
                out=out_flat[:, c0:c0 + wout],
                in_=lchunk.rearrange("b gi f -> b (gi f)")[:, :wout])
            g0 += ng


# ---------------- host side ----------------

_CACHED = {}


def _prep_inputs(inputs):
    f = np.float64
    w_embed = inputs["w_embed"].astype(f)
    wq, wk, wv, wo = (inputs[k].astype(f) for k in ("wq", "wk", "wv", "wo"))
    bq, bk, bv, bo = (inputs[k].astype(f) for k in ("bq", "bk", "bv", "bo"))
    g1, be1 = inputs["g1"].astype(f), inputs["be1"].astype(f)
    g2, be2 = inputs["g2"].astype(f), inputs["be2"].astype(f)
    g3, be3 = inputs["g3"].astype(f), inputs["be3"].astype(f)
    w1, bl1 = inputs["w1"].astype(f), inputs["bl1"].astype(f)
    w2, bl2 = inputs["w2"].astype(f), inputs["bl2"].astype(f)
    dup_pool = inputs["dup_pool"].astype(f)
    dup_bias = inputs["dup_bias"].astype(f)

    sc = 1.0 / np.sqrt(HD)
    bo_eff = bo + wo @ bv
    bl1_eff = bl1 + w1 @ be2
    bl2_eff = bl2 + be2
    dup2 = dup_pool.transpose(1, 0, 2) * g3[:, None, None]  # [D, G, DF]
    db2 = np.concatenate([dup_bias, np.zeros(G * DF - NCLS)])  # [G*DF]
    db2 = db2 + np.einsum("d,gdf->gf", be3, dup_pool).reshape(-1)
    w1T_eff = (w1 * g2[None, :]).T  # [D, FF]

    x = np.ascontiguousarray(inputs["x"].reshape(B, C_IN, S))
    if SP > S:
        x = np.concatenate([x, np.zeros((B, C_IN, SP - S), np.float32)], axis=2)

    np32 = np.float32
    base = {
        "wembT": np.ascontiguousarray(w_embed.T).astype(NP_DT),
        "wkT": np.ascontiguousarray(wk.T).astype(NP_DT),
        "wvT": np.ascontiguousarray(wv.T).astype(NP_DT),
        "wqT": np.ascontiguousarray(wq.T * sc).astype(NP_DT),
        "woT": np.ascontiguousarray(wo.T).astype(NP_DT),
        "w1T": np.ascontiguousarray(w1T_eff).astype(NP_DT),
        "w2T": np.ascontiguousarray(w2.T).astype(NP_DT),
        "dup": np.ascontiguousarray(dup2).astype(ml_dtypes.bfloat16),
        "db2": db2.reshape(G, DF).astype(ml_dtypes.bfloat16),
        "qe": inputs["query_embed"].astype(np32),
        "g1": g1.astype(np32), "be1": be1.astype(np32), "g2": g2.astype(np32),
        "bemb": inputs["b_embed"].astype(np32),
        "bk": bk.astype(np32),
        "bq": (bq * sc).astype(np32),
        "bl1": bl1_eff.astype(np32),
        "bo_row": bo_eff.astype(NP_DT).reshape(1, D),
        "bl2_row": bl2_eff.astype(NP_DT).reshape(1, D),
        "ones_mm": np.ones((1, 128), NP_DT),
        "ones_bf": np.ones((1, 128), ml_dtypes.bfloat16),
    }
    in_maps = []
    for c in range(N_CORES):
        m = dict(base)
        m["x"] = np.ascontiguousarray(x[c * BL:(c + 1) * BL]).astype(NP_DT)
        in_maps.append(m)
    return in_maps


def get_nc():
    if "nc" not in _CACHED:
        _CACHED["nc"] = build_kernel()
    return _CACHED["nc"]


def kernel(**inputs) -> np.ndarray:
    nc = get_nc()
    in_maps = _prep_inputs(inputs)
    res = run_bass_kernel_spmd(nc, in_maps, core_ids=list(range(N_CORES)))
    _CACHED["last_res"] = res
    return np.concatenate([res.results[c]["out"] for c in range(N_CORES)], axis=0)



# revision 3
# speedup vs baseline: 1.0740x; 1.0740x over previous
"""MLDecoder classification head on 8 Trainium2 NeuronCores — v2.

Data-parallel over batch B=64 -> 8 cores x 8 batches, params replicated.
Redesign vs baseline:
  - Wk folded into phase-0 (B = Wk_h^T q_h once); per-batch kT eliminated;
    scoresT = memT . B comes out pre-transposed [s, g] (no PE transposes).
  - No-max softmax (scores bounded ~±5); denominators via ones-column in v;
    normalization fused into the ctx psum->sbuf copy.
  - LN2/LN3 computed in transposed [d, g] space: stats via PE ones-matmuls,
    rsqrt via Ln+Exp (single act table: exp_and_friends group), h stays
    transposed for the grouped FC.
  - Grouped FC swapped: out[f, b] with lhsT=dup (C=8), output written as
    [DF, G, BL] and transposed on host (pure layout transform).
  - DMAs: x on sync queue (contiguous p-major interleave), weights split
    across sync+gpsimd queues, dup prefetched early + tail loaded across
    4 queues in parallel at the end.
Host-side prep is limited to layout transforms and standard parameter
folding (LN gains into adjacent weights, softmax scale into wq, bias
absorption), all O(params).
"""

import numpy as np
import ml_dtypes

import concourse.bass as bass
import concourse.mybir as mybir
import concourse.tile as tile
from concourse import bacc
from concourse.masks import make_identity
from concourse.bass_utils import run_bass_kernel_spmd

# ---------------- problem dims (hardcoded) ----------------
B, C_IN, H, W = 64, 2048, 14, 14
D, FF, G, NCLS, NH = 768, 2048, 100, 9605, 8
DF = 97
HD = D // NH  # 96
S = H * W  # 196
EPS = 1e-5

N_CORES = 8
BL = B // N_CORES  # 8

KC_D = D // 128  # 6
KC_FF = FF // 128  # 16
NJ = C_IN // 128  # 16 interleave factor for x

SBLK_ROWS = [128, S - 128]  # 128, 68
LN768 = float(np.log(768.0))
EPSP = float(768.0 * 768.0 * EPS)

DUP_EARLY = 16  # groups resident through the pipeline
DUP_LATE = G - DUP_EARLY  # loaded across 3 queues at the end

f32 = mybir.dt.float32
bf16 = mybir.dt.bfloat16
fp8 = mybir.dt.float8e4
AF = mybir.ActivationFunctionType
ALU = mybir.AluOpType


def fbc(ap, n, pos=1):
    """Insert a stride-0 (broadcast) free dim of size n at position pos."""
    dims = [list(d) for d in ap.ap]
    dims.insert(pos, [0, n])
    return bass.AP(tensor=ap.tensor, offset=ap.offset, ap=dims)


def fbc_last(ap, n):
    dims = [list(d) for d in ap.ap] + [[0, n]]
    return bass.AP(tensor=ap.tensor, offset=ap.offset, ap=dims)


# ---------------- device kernel ----------------

def build_kernel():
    nc = bacc.Bacc("TRN2", target_bir_lowering=False)

    specs = [
        ("x", (BL, C_IN, S), fp8),
        ("wembT", (128, NJ // 2, KC_D, 2, 128), fp8),
        ("wvT", (128, KC_D, D), bf16),
        ("woT", (HD, NH, D), bf16),
        ("w1T", (128, KC_D, FF), bf16),
        ("w2T", (128, KC_FF, D), bf16),
        ("wqT", (128, KC_D, D), bf16),
        ("wkh", (HD, NH, D), bf16),
        ("dup", (D, G, DF), bf16),
        ("db2T", (DF, G), bf16),
        ("qe", (G, D), f32),
        ("g1col", (128, KC_D), f32), ("be1col", (128, KC_D), f32),
        ("bemb_col", (128, KC_D), f32),
        ("bq_col", (HD, NH), f32),
        ("bo_col", (128, KC_D), f32),
        ("g2col", (128, KC_D), f32),
        ("bl1row", (1, FF), bf16),
        ("cffn2row", (1, D), bf16),
    ]
    hs = {n: nc.dram_tensor(n, shp, dt, kind="ExternalInput") for n, shp, dt in specs}
    hs["out"] = nc.dram_tensor("out", (DF, G, BL), f32, kind="ExternalOutput")

    with tile.TileContext(nc) as tc:
        _body(nc, tc, hs)
    nc.finalize()
    return nc


def _body(nc, tc, hs):
    from contextlib import ExitStack

    def dram(name):
        return hs[name][:]

    def bcast(ap, n_part):
        return bass.AP(tensor=ap.tensor, offset=ap.offset,
                       ap=[[0, n_part]] + list(ap.ap))

    ctx = ExitStack()
    with ctx:
        const = ctx.enter_context(tc.tile_pool(name="const", bufs=1))

        # ---- small constants (tiles now, DMAs issued below) ----
        bemb_col = const.tile([128, KC_D], f32)
        bq_col = const.tile([HD, NH], f32)
        bo_col = const.tile([128, KC_D], f32)
        g2col = const.tile([128, KC_D], f32)
        bl1row = const.tile([1, FF], bf16)
        cffn2row = const.tile([1, D], bf16)
        db2T = const.tile([DF, G], bf16)

        ident = const.tile([128, 128], f32)
        make_identity(nc, ident)

        # Pre-load the one act table that serves every function we use
        # (Exp, Ln, Relu, Copy, Identity): natural_log_exp_and_others = id 6.
        # Without this the insertion pass ping-pongs exp_and_others (0) and
        # natural_log (5) at 1.28us per swap.
        nc.scalar.add_instruction(mybir.InstLoadActFuncSet(
            act_func_set_id=6, name=nc.get_next_instruction_name(),
            engine=mybir.EngineType.Activation))

        ones128 = const.tile([128, 128], bf16)
        nc.vector.memset(ones128, 1.0)
        eps4_t = const.tile([128, 1], f32)
        nc.vector.memset(eps4_t, EPS / 4.0)
        g1col = const.tile([128, KC_D], f32)
        be1col = const.tile([128, KC_D], f32)
        epsp_t = const.tile([128, 1], f32)
        nc.vector.memset(epsp_t, EPSP)
        ln768_t = const.tile([128, 1], f32)
        nc.vector.memset(ln768_t, LN768)

        # ---- big weights in one right-side tagged pool; dup chunks will
        # reuse each weight slot (same tag) once its last reader retires ----
        wbig = ctx.enter_context(tc.tile_pool(name="wbig", bufs=1, side="right"))

        xpool = ctx.enter_context(tc.tile_pool(name="xpool", bufs=2))
        x_view = dram("x").rearrange("b (p j) s -> b p (j s)", p=128)
        x_tiles = {}

        def emit_x(b):
            xt = xpool.tile([128, NJ * S], fp8, tag="x")
            nc.sync.dma_start(out=xt, in_=x_view[b])
            x_tiles[b] = xt

        # sync queue: qe (phase0 LN1) -> x0 -> wembT (embed(0) gate) -> ...
        # gpsimd queue: wqT, wkh (phase0) -> wvT -> woT -> w1T -> w2T -> dupe
        mpool = ctx.enter_context(tc.tile_pool(name="mpool", bufs=3))
        work = ctx.enter_context(tc.tile_pool(name="work", bufs=2))
        p0 = tc.alloc_tile_pool(name="p0", bufs=1)
        qe_sb = p0.tile([G, D], f32)
        emit_x(0)
        wembT = wbig.tile([128, NJ // 2, KC_D, 2, 128], fp8, tag="wembT")
        nc.sync.dma_start(out=wembT[:, :, :3], in_=dram("wembT")[:, :, :3])
        nc.sync.dma_start(out=bemb_col, in_=dram("bemb_col"))
        nc.sync.dma_start(out=qe_sb, in_=dram("qe"))
        nc.sync.dma_start(out=wembT[:, :, 3:], in_=dram("wembT")[:, :, 3:])
        emit_x(1)

        wqT = p0.tile([128, KC_D, D], bf16)
        nc.gpsimd.dma_start(out=wqT, in_=dram("wqT"))
        wkh = p0.tile([HD, NH, D], bf16)
        nc.gpsimd.dma_start(out=wkh, in_=dram("wkh"))
        nc.gpsimd.dma_start(out=g1col, in_=dram("g1col"))
        nc.gpsimd.dma_start(out=be1col, in_=dram("be1col"))
        nc.gpsimd.dma_start(out=bq_col, in_=dram("bq_col"))
        wvT = wbig.tile([128, KC_D, D], bf16, tag="wvT")
        nc.gpsimd.dma_start(out=wvT, in_=dram("wvT"))
        woT = wbig.tile([HD, NH, D], bf16, tag="woT")
        nc.gpsimd.dma_start(out=woT, in_=dram("woT"))
        w1T = wbig.tile([128, KC_D, FF], bf16, tag="w1T")
        nc.gpsimd.dma_start(out=w1T, in_=dram("w1T"))
        w2T = wbig.tile([128, KC_FF, D], bf16, tag="w2T")
        nc.gpsimd.dma_start(out=w2T, in_=dram("w2T"))
        nc.gpsimd.dma_start(out=bo_col, in_=dram("bo_col"))
        nc.gpsimd.dma_start(out=g2col, in_=dram("g2col"))
        nc.gpsimd.dma_start(out=bl1row, in_=dram("bl1row"))
        nc.gpsimd.dma_start(out=cffn2row, in_=dram("cffn2row"))
        nc.gpsimd.dma_start(out=db2T, in_=dram("db2T"))
        B_sb = wbig.tile([128, KC_D // 2, 2, 2, 4 * G], fp8, tag="B")

        tnT32 = const.tile([128, KC_D, G], f32)

        # ---- psum pools (exactly 8 banks) ----
        ps = ctx.enter_context(tc.tile_pool(name="ps", bufs=3, space="PSUM"))
        ps_ctx = ctx.enter_context(tc.tile_pool(name="psctx", bufs=1, space="PSUM"))
        ps_inv = ctx.enter_context(tc.tile_pool(name="psinv", bufs=1, space="PSUM"))
        ps_ln = ctx.enter_context(tc.tile_pool(name="psln", bufs=2, space="PSUM"))

        dup_view = dram("dup").rearrange("(kc p) g f -> p kc g f", p=128)
        dupe = const.tile([128, KC_D, DUP_EARLY, DF], bf16)
        hT = const.tile([128, KC_D, G, BL], bf16)
        dup_chunks = [(0, DUP_EARLY, dupe)]

        def emit_dup_chunk(tag, lo, hi, queue_dmas):
            t = wbig.tile([128, KC_D, hi - lo, DF], bf16, tag=tag)
            for qlo, qhi, q in queue_dmas:
                q.dma_start(out=t[:, :, qlo - lo:qhi - lo, :],
                            in_=dup_view[:, :, qlo:qhi, :])
            dup_chunks.append((lo, hi, t))

        def emit_phase0():
            # LN1 on tgt = 2*qe: LN(2x) == LN(x) with eps/4 (exact), and the
            # g1/be1 affine is applied post-transpose as per-partition scalars.
            tnT = p0.tile([128, KC_D, G], bf16)
            st1 = p0.tile([G, 3, 6], f32)
            qv = qe_sb.rearrange("g (n f) -> g n f", f=256)
            for i in range(3):
                nc.vector.bn_stats(out=st1[:, i, :], in_=qv[:, i, :])
            mv = p0.tile([G, 2], f32)
            nc.vector.bn_aggr(out=mv, in_=st1)
            lnv1 = p0.tile([G, 1], f32)
            nc.scalar.activation(out=lnv1, in_=mv[:, 1:2], func=AF.Ln,
                                 bias=eps4_t[:G], scale=1.0)
            inv1 = p0.tile([G, 1], f32)
            nc.scalar.activation(out=inv1, in_=lnv1, func=AF.Exp,
                                 bias=0.0, scale=-0.5)
            tgt_n = p0.tile([G, D], f32)
            nc.vector.tensor_scalar(out=tgt_n, in0=qe_sb,
                                    scalar1=mv[:, 0:1], scalar2=inv1,
                                    op0=ALU.subtract, op1=ALU.mult)

            # tnT32 / tnT: transposed affine targets [d, g]
            for kc in range(KC_D):
                p_t = ps.tile([128, 512], f32, tag="ps")
                nc.tensor.transpose(p_t[:128, :G], tgt_n[:, kc * 128:(kc + 1) * 128],
                                    ident[:G, :G])
                nc.vector.tensor_scalar(out=tnT32[:, kc, :], in0=p_t[:128, :G],
                                        scalar1=g1col[:, kc:kc + 1],
                                        scalar2=be1col[:, kc:kc + 1],
                                        op0=ALU.mult, op1=ALU.add)
            nc.scalar.activation(out=tnT, in_=tnT32, func=AF.Copy,
                                 bias=0.0, scale=1.0)

            # qT[hd, h, g] = Wq_h tgt_n^T + bq  (scale already folded)
            qT = p0.tile([HD, NH, G], bf16)
            for hh in range(2):
                p_q = ps.tile([128, 4, 128], f32, tag="ps")
                for i in range(4):
                    h = hh * 4 + i
                    for kc in range(KC_D):
                        nc.tensor.matmul(p_q[:HD, i, :G],
                                         wqT[:, kc, h * HD:(h + 1) * HD],
                                         tnT[:, kc, :],
                                         start=(kc == 0), stop=(kc == KC_D - 1))
                for i in range(4):
                    h = hh * 4 + i
                    nc.scalar.activation(out=qT[:, h, :], in_=p_q[:HD, i, :G],
                                         func=AF.Identity, bias=bq_col[:, h:h + 1],
                                         scale=1.0)

            # B[d, kc, h, g] = Wk_h^T q_h  (bk dropped: constant along s
            # shifts every softmax logit of a (g,h) equally -> cancels)
            for dblk in range(KC_D):
                for hh in range(2):
                    p_b = ps.tile([128, 4, 128], f32, tag="ps")
                    for i in range(4):
                        h = hh * 4 + i
                        nc.tensor.matmul(p_b[:, i, :G],
                                         wkh[:, h, dblk * 128:(dblk + 1) * 128],
                                         qT[:, h, :], start=True, stop=True)
                    nc.scalar.activation(
                        out=B_sb[:, dblk // 2, hh, dblk % 2, :].rearrange(
                            "p (h g) -> p h g", g=G),
                        in_=p_b[:, :, :G],
                        func=AF.Copy, bias=0.0, scale=1.0)
            p0.release()

        def emit_embed(b):
            # fp8 DoubleRow: 256-deep contraction per matmul, 0.5 cyc/row
            xt = x_tiles.pop(b)
            memT = mpool.tile([128, KC_D, S], bf16, tag="memT")
            xv = xt.rearrange("p (j two s) -> p j two s", j=NJ // 2, two=2)
            for dblk in range(KC_D):
                p_e = ps.tile([128, 512], f32, tag="ps")
                for j in range(NJ // 2):
                    nc.tensor.matmul(p_e[:, :S],
                                     wembT[:, j, dblk, :, :],
                                     xv[:, j, :, :],
                                     start=(j == 0), stop=(j == NJ // 2 - 1),
                                     perf_mode=mybir.MatmulPerfMode.DoubleRow)
                nc.scalar.activation(out=memT[:, dblk, :], in_=p_e[:, :S],
                                     func=AF.Relu, bias=bemb_col[:, dblk:dblk + 1],
                                     scale=1.0)
            memT8 = mpool.tile([128, 2, KC_D, 128], fp8, tag="memT8")
            nc.scalar.activation(out=memT8[:, 0, :, :], in_=memT[:, :, :128],
                                 func=AF.Copy, bias=0.0, scale=1.0)
            nc.scalar.activation(out=memT8[:, 1, :, :68], in_=memT[:, :, 128:],
                                 func=AF.Copy, bias=0.0, scale=1.0)
            return memT, memT8

        def emit_v(b, memT):
            v_sb = work.tile([128, 2, NH, DF], bf16, tag="v")
            nc.vector.memset(v_sb[:, :, :, HD:DF], 1.0)
            for sblk in range(2):
                rows = SBLK_ROWS[sblk]
                ssl = slice(sblk * 128, sblk * 128 + rows)
                for nch in range(2):
                    p_v = ps.tile([128, 4, HD], f32, tag="ps")
                    for kc in range(KC_D):
                        nc.tensor.matmul(p_v[:rows, :, :], memT[:, kc, ssl],
                                         wvT[:, kc, nch * 384:(nch + 1) * 384],
                                         start=(kc == 0), stop=(kc == KC_D - 1))
                    nc.vector.tensor_copy(
                        out=v_sb[:rows, sblk, nch * 4:(nch + 1) * 4, :HD],
                        in_=p_v[:rows, :, :])
            return v_sb

        def emit_scores(b, memT8):
            # fp8 DoubleRow over d: 3 chunks of 256
            attnT = work.tile([128, 2, NH * G], bf16, tag="attnT")
            for sblk in range(2):
                rows = SBLK_ROWS[sblk]
                for half in range(2):
                    p_s = ps.tile([128, 512], f32, tag="ps")
                    for j in range(KC_D // 2):
                        nc.tensor.matmul(
                            p_s[:rows, :400],
                            memT8[:, sblk, 2 * j:2 * j + 2, :rows],
                            B_sb[:, j, half, :, :],
                            start=(j == 0), stop=(j == KC_D // 2 - 1),
                            perf_mode=mybir.MatmulPerfMode.DoubleRow)
                    nc.scalar.activation(
                        out=attnT[:rows, sblk, half * 400:(half + 1) * 400],
                        in_=p_s[:rows, :400], func=AF.Exp, bias=0.0, scale=1.0)
            return attnT

        def emit_ctx(b, v_sb, attnT):
            ctx_ps = ps_ctx.tile([128, NH, 128], f32, tag="ctxps")
            for h in range(NH):
                for sblk in range(2):
                    rows = SBLK_ROWS[sblk]
                    nc.tensor.matmul(ctx_ps[:DF, h, :G],
                                     v_sb[:rows, sblk, h, :],
                                     attnT[:rows, sblk, h * G:(h + 1) * G],
                                     start=(sblk == 0), stop=(sblk == 1))
            inv_f = one.tile([1, NH * G], f32, tag="invf")
            nc.vector.reciprocal(out=inv_f, in_=ctx_ps[HD:DF, :, :G])
            inv_b = smal.tile([1, NH * G], bf16, tag="invb")
            nc.scalar.activation(out=inv_b, in_=inv_f, func=AF.Copy,
                                 bias=0.0, scale=1.0)
            return ctx_ps, inv_b

        def emit_attn_norm(b, ctx_ps, inv_b):
            ctxT = work.tile([HD, NH, G], bf16, tag="ctxT")
            inv_sb = smal.tile([HD, NH, G], f32, tag="invsb")
            for half in range(2):
                inv_ps = ps_inv.tile([128, 4, 128], f32, tag="invps")
                for i in range(4):
                    h = half * 4 + i
                    nc.tensor.matmul(inv_ps[:HD, i, :G], ones128[0:1, :HD],
                                     inv_b[:, h * G:(h + 1) * G],
                                     start=True, stop=True)
                # HW TensorTensor allows only one PSUM operand: stage the
                # broadcast rows to SBUF first.
                nc.scalar.activation(out=inv_sb[:, half * 4:(half + 1) * 4, :],
                                     in_=inv_ps[:HD, :, :G], func=AF.Copy,
                                     bias=0.0, scale=1.0)
                nc.vector.tensor_mul(ctxT[:, half * 4:(half + 1) * 4, :],
                                     ctx_ps[:HD, half * 4:(half + 1) * 4, :G],
                                     inv_sb[:, half * 4:(half + 1) * 4, :])
            return ctxT

        def emit_ln_stats(src32, tag):
            """Stats for transposed layernorm: PE ones-matmuls on a bf16
            shadow, then the scalar fixup chain through m_r/inv_r rows.
            rsqrt via Ln+Exp (stays in the one preloaded act table)."""
            sh = one.tile([128, KC_D, G], bf16, tag="ln_sh")
            nc.scalar.activation(out=sh, in_=src32, func=AF.Copy,
                                 bias=0.0, scale=1.0)
            sq = one.tile([128, KC_D, G], bf16, tag="ln_sq")
            nc.vector.tensor_mul(sq, sh, sh)
            lps = ps_ln.tile([128, 4, 128], f32, tag="lnps")
            for kc in range(KC_D):
                nc.tensor.matmul(lps[0:1, 0, :G], ones128[:, 0:1], sh[:, kc, :],
                                 start=(kc == 0), stop=(kc == KC_D - 1))
            for kc in range(KC_D):
                nc.tensor.matmul(lps[0:1, 1, :G], ones128[:, 0:1], sq[:, kc, :],
                                 start=(kc == 0), stop=(kc == KC_D - 1))
            st = one.tile([1, 2, G], f32, tag="ln_st")
            nc.vector.tensor_copy(out=st, in_=lps[0:1, 0:2, :G])
            sqm = one.tile([1, G], f32, tag="ln_sqm")
            nc.vector.tensor_mul(sqm, st[:, 0, :], st[:, 0, :])
            varp = one.tile([1, G], f32, tag="ln_varp")
            nc.vector.scalar_tensor_tensor(out=varp, in0=st[:, 1, :], scalar=768.0,
                                           in1=sqm, op0=ALU.mult, op1=ALU.subtract)
            lnv = one.tile([1, G], f32, tag="ln_lnv")
            nc.scalar.activation(out=lnv, in_=varp, func=AF.Ln,
                                 bias=epsp_t[:1], scale=1.0)
            inv_r = smal.tile([1, G], bf16, tag=tag + "_invr")
            nc.scalar.activation(out=inv_r, in_=lnv, func=AF.Exp,
                                 bias=ln768_t[:1], scale=-0.5)
            m_r = smal.tile([1, G], bf16, tag=tag + "_mr")
            nc.vector.tensor_scalar(out=m_r, in0=st[:, 0, :],
                                    scalar1=1.0 / 768.0, scalar2=None,
                                    op0=ALU.mult)
            return lps, m_r, inv_r

        def emit_ln_fin(lns, src32, dst16):
            lps, m_r, inv_r = lns
            nc.tensor.matmul(lps[:, 2, :G], ones128[0:1, :], m_r,
                             start=True, stop=True)
            nc.tensor.matmul(lps[:, 3, :G], ones128[0:1, :], inv_r,
                             start=True, stop=True)
            cen = one.tile([128, KC_D, G], f32, tag="cen")
            nc.vector.tensor_sub(cen, src32, fbc(lps[:, 2, :G], KC_D))
            nc.vector.tensor_mul(dst16, cen, fbc(lps[:, 3, :G], KC_D))

        def emit_oproj(b, ctxT):
            t2T = tpool.tile([128, KC_D, G], f32, tag="t2T")
            for half in range(2):
                p_o = ps.tile([128, 4, 128], f32, tag="ps")
                nblk = 4 if half == 0 else 2
                for i in range(nblk):
                    eblk = half * 4 + i
                    for h in range(NH):
                        nc.tensor.matmul(p_o[:, i, :G],
                                         woT[:, h, eblk * 128:(eblk + 1) * 128],
                                         ctxT[:, h, :],
                                         start=(h == 0), stop=(h == NH - 1))
                for i in range(nblk):
                    eblk = half * 4 + i
                    nc.vector.scalar_tensor_tensor(
                        out=t2T[:, eblk, :], in0=p_o[:, i, :G],
                        scalar=bo_col[:, eblk:eblk + 1], in1=tnT32[:, eblk, :],
                        op0=ALU.add, op1=ALU.add)
            return t2T

        def emit_ffn1(b, ln2T):
            ffT = one.tile([128, KC_FF, G], bf16, tag="ffT")
            for q in range(4):
                p_f = ps.tile([128, 4, 128], f32, tag="ps")
                for i in range(4):
                    fblk = q * 4 + i
                    nc.tensor.matmul(p_f[:, i, :G],
                                     bl1row[:, fblk * 128:(fblk + 1) * 128],
                                     ones128[0:1, :G], start=True, stop=False)
                    for kc in range(KC_D):
                        nc.tensor.matmul(p_f[:, i, :G],
                                         w1T[:, kc, fblk * 128:(fblk + 1) * 128],
                                         ln2T[:, kc, :],
                                         start=False, stop=(kc == KC_D - 1))
                nc.scalar.activation(out=ffT[:, q * 4:(q + 1) * 4, :],
                                     in_=p_f[:, :, :G], func=AF.Relu,
                                     bias=0.0, scale=1.0)
            return ffT

        def emit_ffn2(b, ln2T, ffT):
            t3T = tpool.tile([128, KC_D, G], f32, tag="t2T")
            for half in range(2):
                p_2 = ps.tile([128, 4, 128], f32, tag="ps")
                nblk = 4 if half == 0 else 2
                for i in range(nblk):
                    eblk = half * 4 + i
                    nc.tensor.matmul(p_2[:, i, :G],
                                     cffn2row[:, eblk * 128:(eblk + 1) * 128],
                                     ones128[0:1, :G], start=True, stop=False)
                    for kc in range(KC_FF):
                        nc.tensor.matmul(p_2[:, i, :G],
                                         w2T[:, kc, eblk * 128:(eblk + 1) * 128],
                                         ffT[:, kc, :],
                                         start=False, stop=(kc == KC_FF - 1))
                for i in range(nblk):
                    eblk = half * 4 + i
                    nc.vector.scalar_tensor_tensor(
                        out=t3T[:, eblk, :], in0=ln2T[:, eblk, :],
                        scalar=g2col[:, eblk:eblk + 1], in1=p_2[:, i, :G],
                        op0=ALU.mult, op1=ALU.add)
            return t3T

        # ---------- software-pipelined batch loop ----------
        memTs = {}
        memTs[0] = emit_embed(0)
        emit_phase0()
        tpool = ctx.enter_context(tc.tile_pool(name="tpool", bufs=4))
        one = ctx.enter_context(tc.tile_pool(name="one", bufs=1))
        smal = ctx.enter_context(tc.tile_pool(name="smal", bufs=2))
        nc.gpsimd.dma_start(out=dupe, in_=dup_view[:, :, :DUP_EARLY, :])
        emit_x(2)
        memTs[1] = emit_embed(1)

        state = {}
        stA = {}
        pending_ln3 = None
        last = BL - 1

        def emit_stage2b(b):
            """ln2fin + ffn1 + ffn2 + LN3 stats for batch b (stats from the
            previous iteration, so the fixup chain had a full iteration)."""
            t2T, ln2s = stA.pop(b)
            ln2T = work.tile([128, KC_D, G], bf16, tag="ln2T")
            emit_ln_fin(ln2s, t2T, ln2T)
            ffT = emit_ffn1(b, ln2T)
            t3T = emit_ffn2(b, ln2T, ffT)
            return t3T, emit_ln_stats(t3T, "ln3")

        for b in range(BL):
            if b + 3 < BL:
                emit_x(b + 3)
            if b + 2 <= last:
                memTs[b + 2] = emit_embed(b + 2)
                if b + 2 == last:
                    emit_dup_chunk("wembT", DUP_EARLY, DUP_EARLY + 21,
                                   [(DUP_EARLY, DUP_EARLY + 21, nc.sync)])
            if pending_ln3 is not None:
                ln3s_d, t3T_d, bd = pending_ln3
                emit_ln_fin(ln3s_d, t3T_d, hT[:, :, :, bd])
                pending_ln3 = None
            prev = state.pop(b - 1, None)
            if prev is not None:
                ctxT_p = emit_attn_norm(b - 1, prev[0], prev[1])
            memT, memT8 = memTs.pop(b)
            v_sb = emit_v(b, memT)
            if b == last:
                emit_dup_chunk("wvT", DUP_EARLY + 21, DUP_EARLY + 28,
                               [(DUP_EARLY + 21, DUP_EARLY + 28, nc.sync)])
            if prev is not None:
                t2Tp = emit_oproj(b - 1, ctxT_p)
                stA[b - 1] = (t2Tp, emit_ln_stats(t2Tp, "ln2"))
            attnT = emit_scores(b, memT8)
            if b == last:
                emit_dup_chunk("B", DUP_EARLY + 28, DUP_EARLY + 32,
                               [(DUP_EARLY + 28, DUP_EARLY + 32, nc.sync)])
            if b - 2 in stA:
                t3Tp, ln3s = emit_stage2b(b - 2)
                pending_ln3 = (ln3s, t3Tp, b - 2)
            state[b] = emit_ctx(b, v_sb, attnT)

        # ---------- drain: batches last-1 and last ----------
        if pending_ln3 is not None:
            ln3s_d, t3T_d, bd = pending_ln3
            emit_ln_fin(ln3s_d, t3T_d, hT[:, :, :, bd])
        ctxT_l = emit_attn_norm(last, *state.pop(last))
        t2Tl = emit_oproj(last, ctxT_l)
        stA[last] = (t2Tl, emit_ln_stats(t2Tl, "ln2"))
        e0 = DUP_EARLY + 32
        emit_dup_chunk("woT", e0, e0 + 10, [(e0, e0 + 10, nc.gpsimd)])
        t3T6, ln3s6 = emit_stage2b(last - 1)
        emit_ln_fin(ln3s6, t3T6, hT[:, :, :, last - 1])
        e1 = e0 + 10
        n1a = min(21, (G - e1 + 1) // 2)
        mid = e1 + n1a
        t3T7, ln3s7 = emit_stage2b(last)
        emit_dup_chunk("w1T", e1, mid,
                       [(e1, (e1 + mid) // 2, nc.sync),
                        ((e1 + mid) // 2, mid, nc.gpsimd)])
        emit_ln_fin(ln3s7, t3T7, hT[:, :, :, last])
        emit_dup_chunk("w2T", mid, G,
                       [(mid, (mid + G) // 2, nc.sync),
                        ((mid + G) // 2, G, nc.gpsimd)])

        # ---------- grouped FC: out[f, g, b] ----------
        def dup_src(g):
            for lo, hi, t in dup_chunks:
                if lo <= g < hi:
                    return t[:, :, g - lo, :]
            raise AssertionError(g)

        lsb = const.tile([DF, G, BL], f32)
        out_d = dram("out")
        g0 = 0
        for chunk, ng in enumerate((64, 36)):
            p_g = ps.tile([128, 64, 8], f32, tag="ps")
            for gi in range(ng):
                dsrc = dup_src(g0 + gi)
                for kc in range(KC_D):
                    nc.tensor.matmul(p_g[:DF, gi, :], dsrc[:, kc, :],
                                     hT[:, kc, g0 + gi, :],
                                     start=(kc == 0), stop=(kc == KC_D - 1))
            nc.vector.tensor_add(lsb[:, g0:g0 + ng, :], p_g[:DF, :ng, :],
                                 fbc_last(db2T[:, g0:g0 + ng], BL))
            g0 += ng
        nc.gpsimd.dma_start(out=out_d, in_=lsb)


# ---------------- host side ----------------

_CACHED = {}


def _prep_inputs(inputs):
    f = np.float64
    w_embed = inputs["w_embed"].astype(f)
    wq, wk, wv, wo = (inputs[k].astype(f) for k in ("wq", "wk", "wv", "wo"))
    bq, bv, bo = (inputs[k].astype(f) for k in ("bq", "bv", "bo"))
    g1, be1 = inputs["g1"].astype(f), inputs["be1"].astype(f)
    g2, be2 = inputs["g2"].astype(f), inputs["be2"].astype(f)
    g3, be3 = inputs["g3"].astype(f), inputs["be3"].astype(f)
    w1, bl1 = inputs["w1"].astype(f), inputs["bl1"].astype(f)
    w2, bl2 = inputs["w2"].astype(f), inputs["bl2"].astype(f)
    dup_pool = inputs["dup_pool"].astype(f)
    dup_bias = inputs["dup_bias"].astype(f)

    sc = 1.0 / np.sqrt(HD)
    bo_eff = bo + wo @ bv
    bl1_eff = bl1 + w1 @ be2
    w1T_eff = (w1 * g2[None, :]).T  # [D, FF]
    cffn2 = be2 + bl2
    dup2 = dup_pool.transpose(1, 0, 2) * g3[:, None, None]  # [D, G, DF]
    db2 = np.concatenate([dup_bias, np.zeros(G * DF - NCLS)])
    db2 = db2 + np.einsum("d,gdf->gf", be3, dup_pool).reshape(-1)

    bf = ml_dtypes.bfloat16
    n32 = np.float32

    def colsD(v):  # [D] -> [128, KC_D] with d = kc*128 + p
        return np.ascontiguousarray(v.reshape(KC_D, 128).T).astype(n32)

    def chunkT(m, kc):  # [D_in, E] -> [128, kc, E]
        return np.ascontiguousarray(
            m.reshape(kc, 128, m.shape[1]).transpose(1, 0, 2)).astype(bf)

    def headT(m):  # [D, E] -> [96, NH, E] with d = h*96 + hd
        return np.ascontiguousarray(
            m.reshape(NH, HD, m.shape[1]).transpose(1, 0, 2)).astype(bf)

    x = inputs["x"].reshape(B, C_IN, S)

    base = {
        "wembT": np.ascontiguousarray(
            w_embed.T.reshape(128, NJ // 2, 2, KC_D, 128)
            .transpose(0, 1, 3, 2, 4)).astype(ml_dtypes.float8_e4m3fn),
        "wvT": chunkT(wv.T, KC_D),
        "wqT": chunkT(wq.T * sc, KC_D),
        "w1T": chunkT(w1T_eff, KC_D),
        "w2T": chunkT(w2.T, KC_FF),
        "woT": headT(wo.T),
        "wkh": headT(wk),
        "dup": np.ascontiguousarray(dup2).astype(bf),
        "db2T": np.ascontiguousarray(db2.reshape(G, DF).T).astype(bf),
        "qe": inputs["query_embed"].astype(n32),
        "g1col": colsD(g1), "be1col": colsD(be1),
        "bemb_col": colsD(inputs["b_embed"].astype(f)),
        "bq_col": np.ascontiguousarray(
            (bq * sc).reshape(NH, HD).T).astype(n32),
        "bo_col": colsD(bo_eff),
        "g2col": colsD(g2),
        "bl1row": bl1_eff.reshape(1, FF).astype(bf),
        "cffn2row": cffn2.reshape(1, D).astype(bf),
    }
    in_maps = []
    for c in range(N_CORES):
        m = dict(base)
        m["x"] = np.ascontiguousarray(x[c * BL:(c + 1) * BL]).astype(ml_dtypes.float8_e4m3fn)
        in_maps.append(m)
    return in_maps


def get_nc():
    if "nc" not in _CACHED:
        _CACHED["nc"] = build_kernel()
    return _CACHED["nc"]


def kernel(**inputs) -> np.ndarray:
    nc = get_nc()
    in_maps = _prep_inputs(inputs)
    res = run_bass_kernel_spmd(nc, in_maps, core_ids=list(range(N_CORES)))
    _CACHED["last_res"] = res
    outs = []
    for c in range(N_CORES):
        arr = np.asarray(res.results[c]["out"])  # [DF, G, BL]
        outs.append(arr.transpose(2, 1, 0).reshape(BL, G * DF)[:, :NCLS])
    return np.concatenate(outs, axis=0).astype(np.float32)
